# revision 14
# baseline (speedup 1.0000x reference)
"""AdaProj kernel for 8 TRN2 NeuronCores — v3.

Math (validated vs reference, scale-invariant in W and x):
  out[b,c] = num / sqrt(den * nx2)
  num  = sum_s m_s^2,  m_s = rnw_s * L_s,  L_s[c,b] = W8[c,s,:] . x[b,:]
  den  = num + sum_{s<s'} g2m_ss' * m_s * m_s'
  g2m  = 2*G_ss'*rnw_s*rnw_s'   (per-class cols)
  nx2  = ||x_b||^2  (broadcast over partitions via gpsimd all-reduce)
wt (matmul W) in fp8e3 (x8 host scale); wcm (norm/gram W) in fp16.

Sharding: W split over classes C (125/core); x replicated; out gathered on host.
"""

import numpy as np
import ml_dtypes

import concourse.bacc as bacc
import concourse.bass as bass
import concourse.mybir as mybir
import concourse.tile as tile
from concourse.bass_utils import run_bass_kernel_spmd

B, C, S, D = 256, 1000, 4, 512
NCORES = 8
CS = C // NCORES  # 125
KP = D // 128     # 4

F32 = mybir.dt.float32
F16 = mybir.dt.float16
F8 = mybir.dt.float8e3
AF = mybir.ActivationFunctionType
OP = mybir.AluOpType
RED = bass.bass_isa.ReduceOp

_CACHED = {}


def _emit(nc, pool, psum):
    def st(shape, dtype, name, sp=None):
        p = sp if sp is not None else pool
        return p.tile(shape, dtype, tag=name, name=name)

    # ---- DRAM ----
    wcma_d = nc.dram_tensor("wcma", [CS, 2, D], F16, kind="ExternalInput")
    wcmb_d = nc.dram_tensor("wcmb", [CS, 2, D], F16, kind="ExternalInput")
    xt_d = nc.dram_tensor("xt", [128, KP, B], F16, kind="ExternalInput")
    wta_d = nc.dram_tensor("wta", [128, 2, KP, CS], F8, kind="ExternalInput")
    wtb_d = nc.dram_tensor("wtb", [128, 2, KP, CS], F8, kind="ExternalInput")
    out_d = nc.dram_tensor("out", [CS, B], F16, kind="ExternalOutput")

    # ---- activation table warmup (sqrt table) ----
    warm = st([1, 1], F32, "warm")
    nc.vector.memset(warm[:], 1.0)
    warm2 = st([1, 1], F32, "warm2")
    nc.scalar.activation(warm2[:], warm[:], AF.Abs_reciprocal_sqrt)

    # ---- input DMAs (program order = transfer order on the DMA device) ----
    wcma = st([CS, 2, D], F16, "wcma")  # s0,s1
    wcmb = st([CS, 2, D], F16, "wcmb")  # s2,s3
    xt = st([128, KP, B], F16, "xt")
    wta = st([128, 2, KP, CS], F8, "wta")
    wtb = st([128, 2, KP, CS], F8, "wtb")
    nc.sync.dma_start(wcma[:], wcma_d[:])
    nc.sync.dma_start(xt[:], xt_d[:])
    nc.sync.dma_start(wta[:], wta_d[:])
    nc.sync.dma_start(wcmb[:], wcmb_d[:])
    nc.sync.dma_start(wtb[:], wtb_d[:])

    def wslice(s):
        return wcma[:, s, :] if s < 2 else wcmb[:, s - 2, :]

    # ---- PE warmup: keep the tensor engine continuously busy from t~1us so
    # the p-state ramp reaches full clock before the real matmuls ----
    dz = st([128, B], F16, "dz")
    nc.vector.memset(dz[:], 0.0)
    dzw = st([128, 1], F16, "dzw")
    nc.vector.memset(dzw[:], 0.0)
    dps = st([1, B], F32, "dps", psum)
    for _ in range(18):
        nc.tensor.matmul(dps[:], dzw[:], dz[:], start=True, stop=True)

    # ---- PE: L matmuls only, s-major, back-to-back ----
    Lp = [st([CS, B], F32, f"L{s}", psum) for s in range(S)]
    for s in range(S):
        wt_s = wta[:, s, :, :] if s < 2 else wtb[:, s - 2, :, :]
        for k in range(KP):
            nc.tensor.matmul(
                Lp[s][:], wt_s[:, k, :], xt[:, k, :],
                start=(k == 0), stop=(k == KP - 1),
            )

    # ---- W norms on Act (square+accum), incremental rsqrts ----
    ns = st([CS, S], F32, "ns")
    rnw = st([CS, S], F32, "rnw")
    sq_scr = st([CS, S, D], F32, "sq_scr")
    for s in range(2):
        nc.scalar.activation(
            sq_scr[:, s, :], wcma[:, s, :], AF.Square,
            accum_out=ns[:, s:s + 1],
        )
    nc.scalar.activation(rnw[:, 0:2], ns[:, 0:2], AF.Abs_reciprocal_sqrt)
    nc.scalar.activation(
        sq_scr[:, 2, :], wcmb[:, 0, :], AF.Square, accum_out=ns[:, 2:3],
    )
    nc.scalar.activation(rnw[:, 2:3], ns[:, 2:3], AF.Abs_reciprocal_sqrt)
    nc.scalar.activation(
        sq_scr[:, 3, :], wcmb[:, 1, :], AF.Square, accum_out=ns[:, 3:4],
    )
    nc.scalar.activation(rnw[:, 3:4], ns[:, 3:4], AF.Abs_reciprocal_sqrt)

    # ---- gram via fused product+accum (stt): DVE 3 pairs, Pool 3 pairs ----
    # g6 cols: 0=(0,1) 1=(1,2) 2=(0,2) 3=(0,3) 4=(1,3) 5=(2,3)
    g6 = st([CS, 6], F32, "g6")
    gscr = st([CS, 6, D], F16, "gscr")
    one = st([CS, 1], F32, "one")
    nc.vector.memset(one[:], 1.0)
    g2m = st([CS, 6], F32, "g2m")
    rr = st([CS, 6], F32, "rr")

    def gram(engine, j, s, s2):
        engine.tensor_tensor(gscr[:, j, :], wslice(s), wslice(s2), OP.mult)

    def g2m_col(j):
        nc.vector.scalar_tensor_tensor(
            out=g2m[:, j:j + 1], in0=g6[:, j:j + 1], scalar=2.0,
            in1=rr[:, j:j + 1], op0=OP.mult, op1=OP.mult,
        )

    # DVE early: (0,1) from wcma, then x-path while waiting wcmb
    gram(nc.vector, 0, 0, 1)
    xsq = st([128, KP, B], F16, "xsq")
    nc.vector.tensor_tensor(xsq[:], xt[:], xt[:], OP.mult)
    xf2 = st([128, 2, B], F16, "xf2")
    nc.vector.tensor_tensor(xf2[:], xsq[:, 0:2, :], xsq[:, 2:4, :], OP.add)
    nx2 = st([128, B], F32, "nx2")
    nc.vector.tensor_tensor(nx2[:], xf2[:, 0, :], xf2[:, 1, :], OP.add)
    nx2bc = st([128, B], F32, "nx2bc")
    nc.gpsimd.partition_all_reduce(nx2bc[:], nx2[:], channels=128, reduce_op=RED.add)
    # rr cols as rnw becomes ready
    nc.vector.tensor_tensor(rr[:, 0:2], rnw[:, 0:2], rnw[:, 1:3], OP.mult)
    nc.vector.tensor_tensor(rr[:, 2:3], rnw[:, 0:1], rnw[:, 2:3], OP.mult)
    g2m_col(0)
    gram(nc.vector, 1, 1, 2)
    gram(nc.vector, 3, 0, 3)
    # Pool: remaining gram products
    gram(nc.gpsimd, 4, 1, 3)
    gram(nc.gpsimd, 5, 2, 3)
    gram(nc.gpsimd, 2, 0, 2)
    # reduces: DVE tr for slots 0:3; Act copy-accum for 3,4,5
    nc.vector.tensor_reduce(g6[:, 0:3], gscr[:, 0:3, :], mybir.AxisListType.X, OP.add)
    gacc = st([CS, 3, D], F32, "gacc")
    for j in range(3, 6):
        nc.scalar.activation(
            gacc[:, j - 3, :], gscr[:, j, :], AF.Copy, accum_out=g6[:, j:j + 1],
        )
    nc.vector.tensor_scalar_mul(rr[:, 3:6], rnw[:, 0:3], rnw[:, 3:4])
    g2m_col(1)
    g2m_col(3)
    g2m_col(4)
    g2m_col(5)
    g2m_col(2)

    # ---- m_s = rnw_s * L_s on Act (PSUM in); m3 separate tile (no false dep) --
    m012 = st([CS, 3, B], F16, "m012")
    for s in range(3):
        nc.scalar.mul(m012[:, s, :], Lp[s][:], rnw[:, s:s + 1])
    m3 = st([CS, B], F16, "m3")
    nc.scalar.mul(m3[:], Lp[3][:], rnw[:, 3:4])

    # ---- DVE chains (by readiness) ----
    # q = g2m(0,3)*m0 + g2m(1,3)*m1 + g2m(2,3)*m2
    qs = st([CS, 3, B], F16, "qs")
    nc.vector.tensor_scalar_mul(qs[:, 0, :], m012[:, 0, :], g2m[:, 3:4])
    nc.vector.tensor_scalar_mul(qs[:, 1, :], m012[:, 1, :], g2m[:, 4:5])
    nc.vector.tensor_scalar_mul(qs[:, 2, :], m012[:, 2, :], g2m[:, 5:6])
    q = st([CS, B], F16, "q")
    nc.vector.tensor_tensor(q[:], qs[:, 0, :], qs[:, 1, :], OP.add)
    nc.vector.tensor_tensor(q[:], q[:], qs[:, 2, :], OP.add)

    cp = st([CS, 3, B], F16, "cp")  # products (0,1),(1,2) then (0,2)
    nc.vector.tensor_tensor(cp[:, 0:2, :], m012[:, 0:2, :], m012[:, 1:3, :], OP.mult)
    nc.vector.tensor_tensor(cp[:, 2:3, :], m012[:, 0:1, :], m012[:, 2:3, :], OP.mult)
    cs_ = st([CS, 3, B], F16, "cs_")
    nc.vector.tensor_scalar_mul(cs_[:, 0, :], cp[:, 0, :], g2m[:, 0:1])
    nc.vector.tensor_scalar_mul(cs_[:, 1, :], cp[:, 1, :], g2m[:, 1:2])
    nc.vector.tensor_scalar_mul(cs_[:, 2, :], cp[:, 2, :], g2m[:, 2:3])
    den_pre = st([CS, B], F16, "den_pre")
    nc.vector.tensor_tensor(den_pre[:], cs_[:, 0, :], cs_[:, 1, :], OP.add)
    nc.vector.tensor_tensor(den_pre[:], den_pre[:], cs_[:, 2, :], OP.add)

    # Act: tp = m^2; num via DVE adds
    tp = st([CS, 3, B], F16, "tp")
    nc.scalar.activation(tp[:], m012[:], AF.Square)
    tp3 = st([CS, B], F16, "tp3")
    nc.scalar.activation(tp3[:], m3[:], AF.Square)
    n01 = st([CS, B], F16, "n01")
    nc.vector.tensor_tensor(n01[:], tp[:, 0, :], tp[:, 1, :], OP.add)
    n23 = st([CS, B], F16, "n23")
    nc.vector.tensor_tensor(n23[:], tp[:, 2, :], tp3[:], OP.add)
    num = st([CS, B], F16, "num")
    nc.vector.tensor_tensor(num[:], n01[:], n23[:], OP.add)

    # ---- tail ----
    T = st([CS, B], F16, "T")
    nc.vector.tensor_tensor(T[:], m3[:], q[:], OP.mult)
    den = st([CS, B], F16, "den")
    nc.vector.tensor_tensor(den[:], den_pre[:], T[:], OP.add)
    dnum = st([CS, B], F16, "dnum")
    nc.vector.tensor_tensor(dnum[:], den[:], num[:], OP.add)
    denx = st([CS, B], F16, "denx")
    nc.vector.tensor_tensor(denx[:], dnum[:], nx2bc[:CS, :], OP.mult)
    srd = st([CS, B], F16, "srd")
    nc.scalar.activation(srd[:], denx[:], AF.Abs_reciprocal_sqrt)
    ot = st([CS, B], F16, "ot")
    nc.vector.tensor_tensor(ot[:], num[:], srd[:], OP.mult)
    nc.sync.dma_start(out_d[:], ot[:])


def _build_nc():
    nc = bacc.Bacc(
        "TRN2", target_bir_lowering=False, debug=False,
        enable_asserts=False, num_devices=NCORES,
    )
    with tile.TileContext(nc) as tc:
        with (
            tc.tile_pool(name="main", bufs=1) as pool,
            tc.tile_pool(name="psum", bufs=1, space="PSUM") as psum,
        ):
            _emit(nc, pool, psum)
    nc.compile()
    return nc


def _get_nc():
    if "nc" not in _CACHED:
        _CACHED["nc"] = _build_nc()
    return _CACHED["nc"]


def _make_in_maps(x, W):
    x = np.ascontiguousarray(np.asarray(x, dtype=np.float32))
    W = np.ascontiguousarray(np.asarray(W, dtype=np.float32))
    f8 = ml_dtypes.float8_e3m4
    xT = x.T.astype(np.float16)  # [D, B]
    xt = np.ascontiguousarray(xT.reshape(KP, 128, B).transpose(1, 0, 2))
    in_maps = []
    for i in range(NCORES):
        Ws = np.ascontiguousarray(W[i * CS:(i + 1) * CS] * 8.0)  # [CS, S, D]
        W16 = Ws.astype(np.float16)
        W8 = Ws.astype(f8)
        # wt8 [p, s, k, c] = W8[c, s, k*128+p]
        wt8 = np.ascontiguousarray(
            W8.astype(np.float32).reshape(CS, S, KP, 128).transpose(3, 1, 2, 0)
        ).astype(f8)
        in_maps.append({
            "wcma": np.ascontiguousarray(W16[:, 0:2]),
            "wcmb": np.ascontiguousarray(W16[:, 2:4]),
            "xt": xt,
            "wta": np.ascontiguousarray(wt8[:, 0:2]),
            "wtb": np.ascontiguousarray(wt8[:, 2:4]),
        })
    return in_maps


def run(x, W, trace=False):
    nc = _get_nc()
    in_maps = _make_in_maps(x, W)
    res = run_bass_kernel_spmd(
        nc, in_maps, core_ids=list(range(NCORES)), trace=trace
    )
    shards = [res.results[i]["out"].astype(np.float32) for i in range(NCORES)]
    out = np.concatenate([s.T for s in shards], axis=1)  # [B, C]
    return np.ascontiguousarray(out.astype(np.float32)), res


def kernel(x, W):
    out, _ = run(x, W, trace=False)
    return out


# revision 15
# speedup vs baseline: 1.0664x; 1.0664x over previous
"""AdaProj kernel for 8 TRN2 NeuronCores.

Math reduction (validated vs reference to ~4e-6 max rel err in f32):
  out[b,c] = rnx_b * num / sqrt(den)
  num = sum_s (rnw_s L_s)^2
  den = num + sum_{s<s'} g2m_ss' * (m_s * m_s'),  m_s = rnw_s * L_s
  g2m = 2*Graw_ss'*rnw_s*rnw_s'  (per-class scalars)
  L_s[c,b] = W[c,s,:] . x[b,:]  (raw matmul), rnw = 1/||W_cs||, rnx = 1/||x_b||
This removes the [B,C,D] intermediate of the reference entirely.

Sharding: W split over classes C (125/core); x replicated. No collectives —
host concatenates the per-core [125, 256] outputs.
"""

import numpy as np
import ml_dtypes

import concourse.bacc as bacc
import concourse.bass as bass
import concourse.mybir as mybir
import concourse.tile as tile
from concourse.bass_utils import run_bass_kernel_spmd

B, C, S, D = 256, 1000, 4, 512
NCORES = 8
CS = C // NCORES  # 125 classes per core
R = CS * S        # 500 W rows per core
KP = D // 128     # 4 contraction chunks
PAIRS = [(0, 1), (0, 2), (0, 3), (1, 2), (1, 3), (2, 3)]

F32 = mybir.dt.float32
BF16 = mybir.dt.bfloat16
FP16 = mybir.dt.float16
AF = mybir.ActivationFunctionType
OP = mybir.AluOpType

_CACHED = {}


def _emit_body(nc, pool, psum, xT_d, wT_d, wcm_d, out_d, it, TIN, parts="all"):
    p = f"i{it}_"

    def st(shape, dtype, name, space_pool=None):
        sp = space_pool if space_pool is not None else pool
        return sp.tile(shape, dtype, tag=p + name, name=p + name)

    # ---------- activation-table warmup (sqrt_and_others covers all) ----
    warm = st([1, 1], F32, "warm")
    nc.vector.memset(warm[:], 1.0)
    warm3 = st([1, 1], F32, "warm3")
    nc.scalar.activation(warm3[:], warm[:], AF.Abs_reciprocal_sqrt)

    # ---------- inputs (separate tiles so tile-granular deps don't chain) --
    xt = st([128, KP, B], TIN, "xt")
    wt = st([128, KP, R], TIN, "wt")
    wcmA = st([CS, 2, D], TIN, "wcmA")  # s = 0,1
    wcmB = st([CS, 2, D], TIN, "wcmB")  # s = 2,3
    nc.sync.dma_start(wcmA[:, :, :], wcm_d[:, 0:2, :])
    nc.sync.dma_start(
        wt[:, :, :], wT_d[:, :].rearrange("(k p) r -> p k r", p=128)
    )
    nc.sync.dma_start(
        xt[:, :, :], xT_d[:, :].rearrange("(k p) b -> p k b", p=128)
    )
    nc.sync.dma_start(wcmB[:, :, :], wcm_d[:, 2:4, :])

    def wslice(s):
        return wcmA[:, s, :] if s < 2 else wcmB[:, s - 2, :]

    # ---------- W norms, first half (s=0,1) ----------
    s2a = st([CS, 2], F32, "s2a")
    s2b = st([CS, 2], F32, "s2b")
    sq_scr = st([CS, S, D], F32, "sq_scr")
    rnwa = st([CS, 2], F32, "rnwa")
    rnwb = st([CS, 2], F32, "rnwb")
    for s in range(2):
        nc.scalar.activation(
            sq_scr[:, s, :], wcmA[:, s, :], AF.Square,
            accum_out=s2a[:, s:s + 1],
        )
    nc.scalar.activation(rnwa[:], s2a[:], AF.Abs_reciprocal_sqrt)

    def rnw_col(s):
        return rnwa[:, s:s + 1] if s < 2 else rnwb[:, s - 2:s - 1]

    # ---------- main matmuls (s-major so L_s completes incrementally) ----
    Lp = [st([CS, B], F32, f"L{s}", psum) for s in range(S)]
    last_L_mm = None
    for s in range(S):
        for k in range(KP):
            last_L_mm = nc.tensor.matmul(
                Lp[s][:],
                wt[:, k, s * CS:(s + 1) * CS],
                xt[:, k, :],
                start=(k == 0), stop=(k == KP - 1),
            )

    # ---------- W norms, second half (s=2,3) ----------
    for s in range(2):
        nc.scalar.activation(
            sq_scr[:, 2 + s, :], wcmB[:, s, :], AF.Square,
            accum_out=s2b[:, s:s + 1],
        )
    nc.scalar.activation(rnwb[:], s2b[:], AF.Abs_reciprocal_sqrt)

    # ---------- m_s = rnw_s * L_s on ScalarE ----------
    m = [st([CS, B], TIN, f"m{s}") for s in range(S)]
    m_insts = []
    for s in range(S):
        m_insts.append(nc.scalar.mul(m[s][:], Lp[s][:], rnw_col(s)))

    # ---------- Gram cross products (pair (0,1) can run earliest) --------
    prod1 = st([CS, 3, D], TIN, "prod1")  # (0,1),(1,2),(2,3)
    nc.vector.tensor_tensor(prod1[:, 0, :], wcmA[:, 0, :], wcmA[:, 1, :], OP.mult)
    nc.vector.tensor_tensor(prod1[:, 1, :], wcmA[:, 1, :], wcmB[:, 0, :], OP.mult)
    nc.vector.tensor_tensor(prod1[:, 2, :], wcmB[:, 0, :], wcmB[:, 1, :], OP.mult)
    prod2 = st([CS, 2, D], TIN, "prod2")  # (0,2),(1,3)
    nc.vector.tensor_tensor(prod2[:], wcmA[:, :, :], wcmB[:, :, :], OP.mult)
    prod3 = st([CS, 1, D], TIN, "prod3")  # (0,3)
    nc.gpsimd.tensor_tensor(prod3[:, 0, :], wcmA[:, 0, :], wcmB[:, 1, :], OP.mult)
    xsq = st([128, KP, B], TIN, "xsq")
    nc.gpsimd.tensor_tensor(xsq[:], xt[:], xt[:], OP.mult)

    gr1 = st([CS, 3], F32, "gr1")
    gr2a_t = st([CS, 1], F32, "gr2a_t")
    gr2b_t = st([CS, 1], F32, "gr2b_t")
    gr3 = st([CS, 1], F32, "gr3")
    red_scr = st([CS, 3, D], F32, "red_scr")
    nc.vector.tensor_reduce(gr1[:], prod1[:], mybir.AxisListType.X, OP.add)
    for j, grt in enumerate([gr2a_t, gr2b_t]):
        cp = nc.scalar.activation(
            red_scr[:, j, :], prod2[:, j, :], AF.Copy,
            accum_out=grt[:],
        )
        if j == 0:
            bass._add_dep_helper(
                cp.ins, m_insts[-1].ins, sync=False,
                reason="m copies go first on ScalarE",
            )
    nc.scalar.activation(
        red_scr[:, 2, :], prod3[:, 0, :], AF.Copy,
        accum_out=gr3[:, 0:1],
    )

    # ---------- epilogue: q/num path (V-queue ordered by readiness) ------
    q = [st([CS, B], TIN, f"q{s}") for s in range(S)]
    n01 = st([CS, B], TIN, "n01")
    n23 = st([CS, B], TIN, "n23")
    num = st([CS, B], TIN, "num")
    ps = [st([CS, B], TIN, f"p{i}") for i in range(6)]
    # work that only needs m0/m1 first
    nc.vector.tensor_tensor(q[0][:], m[0][:], m[0][:], OP.mult)
    nc.gpsimd.tensor_tensor(q[1][:], m[1][:], m[1][:], OP.mult)
    nc.vector.tensor_tensor(ps[0][:], m[0][:], m[1][:], OP.mult)
    nc.vector.tensor_tensor(n01[:], q[0][:], q[1][:], OP.add)
    # then m2-dependent, then m3-dependent
    nc.gpsimd.tensor_tensor(q[2][:], m[2][:], m[2][:], OP.mult)
    nc.vector.tensor_tensor(ps[1][:], m[0][:], m[2][:], OP.mult)
    nc.gpsimd.tensor_tensor(ps[3][:], m[1][:], m[2][:], OP.mult)
    nc.vector.tensor_tensor(q[3][:], m[3][:], m[3][:], OP.mult)
    nc.vector.tensor_tensor(n23[:], q[2][:], q[3][:], OP.add)
    nc.vector.tensor_tensor(num[:], n01[:], n23[:], OP.add)
    nc.gpsimd.tensor_tensor(ps[2][:], m[0][:], m[3][:], OP.mult)
    nc.gpsimd.tensor_tensor(ps[4][:], m[1][:], m[3][:], OP.mult)
    nc.vector.tensor_tensor(ps[5][:], m[2][:], m[3][:], OP.mult)

    # per-pair coefficients: t6 cols in PAIRS order, g2m split by readiness
    t6 = st([CS, 6], F32, "t6")
    for i, (s, sp) in enumerate(PAIRS):
        nc.vector.tensor_tensor(t6[:, i:i + 1], rnw_col(s), rnw_col(sp), OP.mult)
    g2mA = st([CS, 3], F32, "g2mA")  # shift-1 pairs (0,1),(1,2),(2,3)
    g2mB0 = st([CS, 1], F32, "g2mB0")  # (0,2)
    g2mB1 = st([CS, 1], F32, "g2mB1")  # (1,3)
    g2mB2 = st([CS, 1], F32, "g2mB2")  # (0,3)
    S1_IDX = [0, 3, 5]  # PAIRS indices of (0,1),(1,2),(2,3)
    S23_IDX = [1, 4, 2]  # (0,2),(1,3),(0,3)
    for j, i in enumerate(S1_IDX):
        nc.vector.scalar_tensor_tensor(
            out=g2mA[:, j:j + 1], in0=gr1[:, j:j + 1], scalar=2.0,
            in1=t6[:, i:i + 1], op0=OP.mult, op1=OP.mult,
        )
    # chain A: ts pre-scales (2x mode, independent of num) + adds
    cpA = [st([CS, B], TIN, f"cpA{j}") for j in range(3)]
    for j, i in enumerate(S1_IDX):
        nc.vector.tensor_scalar_mul(cpA[j][:], ps[i][:], g2mA[:, j:j + 1])
    accA = [st([CS, B], TIN, f"accA{j}") for j in range(3)]
    nc.vector.tensor_tensor(accA[0][:], num[:], cpA[0][:], OP.add)
    nc.vector.tensor_tensor(accA[1][:], cpA[1][:], cpA[2][:], OP.add)

    nc.vector.scalar_tensor_tensor(
        out=g2mB0[:], in0=gr2a_t[:], scalar=2.0,
        in1=t6[:, 1:2], op0=OP.mult, op1=OP.mult,
    )
    nc.vector.scalar_tensor_tensor(
        out=g2mB1[:], in0=gr2b_t[:], scalar=2.0,
        in1=t6[:, 4:5], op0=OP.mult, op1=OP.mult,
    )
    nc.vector.scalar_tensor_tensor(
        out=g2mB2[:], in0=gr3[:, 0:1], scalar=2.0,
        in1=t6[:, 2:3], op0=OP.mult, op1=OP.mult,
    )
    cpB = [st([CS, B], TIN, f"cpB{j}") for j in range(3)]
    nc.vector.tensor_scalar_mul(cpB[0][:], ps[1][:], g2mB0[:])
    nc.vector.tensor_scalar_mul(cpB[1][:], ps[4][:], g2mB1[:])
    nc.vector.tensor_scalar_mul(cpB[2][:], ps[2][:], g2mB2[:])
    accB = [st([CS, B], TIN, f"accB{j}") for j in range(3)]
    nc.vector.tensor_tensor(accB[1][:], cpB[0][:], cpB[1][:], OP.add)

    # ---------- x norms -> rnx broadcast (consumed late) ----------
    ones = st([128, 1], TIN, "ones")
    nc.vector.memset(ones[:], 1.0)
    nx_ps = st([1, B], F32, "nx", psum)
    for k in range(KP):
        mm = nc.tensor.matmul(
            nx_ps[:], ones[:], xsq[:, k, :],
            start=(k == 0), stop=(k == KP - 1),
        )
        if k == 0:
            bass._add_dep_helper(
                mm.ins, last_L_mm.ins, sync=False,
                reason="keep PE on the L matmuls until they finish",
            )
    rnx_row = st([1, B], F32, "rnx_row")
    nc.scalar.activation(rnx_row[:], nx_ps[:], AF.Abs_reciprocal_sqrt)
    ones_row = st([1, 128], F32, "ones_row")
    nc.vector.memset(ones_row[:], 1.0)
    rnx_ps = st([CS, B], F32, "rnx_bc", psum)
    nc.tensor.matmul(rnx_ps[:], ones_row[:, :CS], rnx_row[:], start=True, stop=True)

    # ---------- tail (independent half-chains, per-half out DMA) ---------
    u = st([CS, B], F32, "u")
    nc.vector.tensor_tensor(u[:], num[:], rnx_ps[:], OP.mult)
    H = B // 2
    for h in range(2):
        hs = slice(h * H, (h + 1) * H)
        a2_h = st([CS, H], TIN, f"a2_{h}")
        b2_h = st([CS, H], TIN, f"b2_{h}")
        den_h = st([CS, H], F32, f"den{h}")
        srd_h = st([CS, H], F32, f"srd{h}")
        ot_h = st([CS, H], FP16, f"ot{h}")
        nc.vector.tensor_tensor(a2_h[:], accA[0][:, hs], accA[1][:, hs], OP.add)
        nc.vector.tensor_tensor(b2_h[:], accB[1][:, hs], cpB[2][:, hs], OP.add)
        nc.vector.tensor_tensor(den_h[:], a2_h[:], b2_h[:], OP.add)
        nc.scalar.activation(srd_h[:], den_h[:], AF.Abs_reciprocal_sqrt)
        nc.vector.tensor_tensor(ot_h[:], u[:, hs], srd_h[:], OP.mult)
        nc.sync.dma_start(out_d[:, hs], ot_h[:])


def _build_nc(use_bf16=True, n_iter=1):
    TIN = FP16 if use_bf16 else F32
    nc = bacc.Bacc(
        "TRN2",
        target_bir_lowering=False,
        debug=False,
        enable_asserts=False,
        num_devices=NCORES,
    )
    xT_d = nc.dram_tensor("xT", [D, B], TIN, kind="ExternalInput")
    wT_d = nc.dram_tensor("wT", [D, R], TIN, kind="ExternalInput")
    wcm_d = nc.dram_tensor("wcm", [CS, S, D], TIN, kind="ExternalInput")
    out_d = nc.dram_tensor("out", [CS, B], FP16, kind="ExternalOutput")

    with tile.TileContext(nc) as tc:
        with (
            tc.tile_pool(name="main", bufs=1) as pool,
            tc.tile_pool(name="psum", bufs=1, space="PSUM") as psum,
        ):
            for it in range(n_iter):
                _emit_body(nc, pool, psum, xT_d, wT_d, wcm_d, out_d, it, TIN)

    nc.compile()
    return nc


def _get_nc():
    if "nc" not in _CACHED:
        _CACHED["nc"] = _build_nc()
    return _CACHED["nc"]


def _make_in_maps(x, W, use_bf16=True):
    x = np.ascontiguousarray(np.asarray(x, dtype=np.float32))
    W = np.ascontiguousarray(np.asarray(W, dtype=np.float32))
    tin = np.float16 if use_bf16 else np.float32
    xT = np.ascontiguousarray(x.T.astype(tin))  # [D, B]
    in_maps = []
    for i in range(NCORES):
        Ws = W[i * CS:(i + 1) * CS].astype(tin)  # [CS, S, D]
        wT = np.ascontiguousarray(Ws.transpose(2, 1, 0).reshape(D, R))
        wcm = np.ascontiguousarray(Ws)  # [CS, S, D]
        in_maps.append({"xT": xT, "wT": wT, "wcm": wcm})
    return in_maps


def run(x, W, trace=False):
    nc = _get_nc()
    in_maps = _make_in_maps(x, W)
    res = run_bass_kernel_spmd(
        nc, in_maps, core_ids=list(range(NCORES)), trace=trace
    )
    shards = [res.results[i]["out"].astype(np.float32) for i in range(NCORES)]
    out = np.concatenate([s.T for s in shards], axis=1)  # [B, C]
    return np.ascontiguousarray(out.astype(np.float32)), res


def kernel(x, W):
    out, _ = run(x, W, trace=False)
    return out
